# revision 13
# baseline (speedup 1.0000x reference)
"""CRF forward (log-partition) kernel for Trainium2, 8 NeuronCores.

Problem: T=16384 steps, NT=1024 tags.
  alpha_0 = strans + emit[0]
  alpha_t[k] = emit[t,k] + logsumexp_j(alpha_{t-1}[j] + trans[j,k])
  out = logsumexp(alpha_{T-1} + etrans) - gold_path_score

Algorithm (v2 — see kernel_baseline.py for the v1 derivation):
  Work in exp space: with Mc[k]=max_j trans[j,k], Ehat=exp(trans-Mc) in (0,1],
  ghat_t = exp(emit[t]+Mc-mu_t), mu_t = max_k(emit[t]+Mc) + RBAR, the scan is a
  LINEAR recurrence  b_t = ghat_t * (Ehat^T b_{t-1})  whose scalar offsets are
  tracked exactly on the host.  2048 chains of L=8 steps; chain i's taus 1..8
  process steps 8i..8i+7 UNIFORMLY (no special chain).  The tau-1 moving
  operand is the G1 tile itself (seed for chain i = ghat[8i], a free-standing
  proxy for b_{8i-1}; positive-matrix products contract direction error
  ~20-30x/step so it is washed out by the chunk's end).  Chain 0's "step 0"
  is fake; its output is DISCARDED and the host instead computes alpha_7
  exactly (7 fp64 LSE steps, 0.04% of the work) to anchor the telescope:
    T_0 = LSE(alpha_7);  T_i = T_{i-1} + P_i + log S(v_i) - log S(u_i),
    P_i = sum(mu[8i:8i+8]);  u_i = e4m3 ghat[8i] column sums (host);
    v_i = dumped end-of-chunk sums (device).
    logZ = T_2047 - log S(v_2047) + log(v_2047 . exp(etrans)).

  Speed: fp8 DoubleRow matmuls contract K=256/instruction; per micro-step,
  8 output blocks x 4 pair-matmuls of [128x(2x128)] e4m3 weights against the
  [128x(2x256)] moving b tile (e4m3 G1 at tau 1, e5m2 after — the per-chain
  scale drifts ~10 logs over a chunk), fp32 PSUM, then 8 DVE multiplies by
  e4m3 ghat writing the next e5m2 b tile.  256 chains/core x 8 cores, W=8
  micro-steps, no inter-core communication.  b tiles split into two
  half-tiles (j-blocks 0-3 / 4-7) so the next micro-step's first matmuls
  only wait on half the DVE writes.

  Trace-driven startup/tail (v2): the HAM clock gate keeps the PE at 1.2GHz
  until ~3.4us of sustained matmul activity, and the runtime preamble +
  input DMA (~1.5MB critical) take ~13us before real matmuls can start.  So:
  (a) NWARM dummy bf16 matmuls issue first on the tensor queue to warm the
  HAM while DMA streams; (b) DMA triggers are need-ordered round-robin
  across the sync/scalar/gpsimd queues (~128KB each, ~1us/queue/item) so
  EH0+G1a land first; (c) the end-of-chunk dump is split into 4x64KB DMAs
  that fire as their two DVE multiplies complete.
"""

import numpy as np

T, NT = 16384, 1024
NCORES = 8
CH = 256            # chains per core
L = 16384 // (NCORES * CH)   # chunk length = 8
W = L               # micro-steps
V_TAU = W
RBAR = 1.0          # per-step growth fold-in; centers the per-chain scale
                    # drift (measured [3.9e-3, 64] over a chunk) in e5m2 range
NWARM = 8           # HAM-warmup matmuls (FD=512, ~450ns each cold) during DMA

_CACHE = {}


def _build_nc():
    import concourse.bass as bass
    import concourse.mybir as mybir
    import concourse.tile as tile
    from concourse import bacc

    nc = bacc.Bacc("TRN2", target_bir_lowering=False, debug=False,
                   num_devices=NCORES)
    bf16 = mybir.dt.bfloat16
    f32 = mybir.dt.float32
    f8w = mybir.dt.float8e4      # weights + ghat: e4m3 precision
    f8b = mybir.dt.float8e5      # moving b: e5m2 range
    DR = mybir.MatmulPerfMode.DoubleRow

    EH = nc.dram_tensor("ehat", [128, 8, 8, 128], f8w, kind="ExternalInput")
    GH = nc.dram_tensor("ghat", [128, W, 8 * CH], f8w, kind="ExternalInput")
    DV = nc.dram_tensor("dv", [4, 128, 2 * CH], f8b, kind="ExternalOutput")

    with tile.TileContext(nc) as tc:
        with (
            tc.tile_pool(name="const", bufs=1) as const,
            tc.tile_pool(name="bpool", bufs=2) as bpool,
            tc.tile_pool(name="gpool", bufs=4) as gpool,
            tc.tile_pool(name="psum", bufs=6, space="PSUM") as psum,
            tc.tile_pool(name="wpsum", bufs=2, space="PSUM") as wpsum,
        ):
            # --- HAM warmup: dummy bf16 matmuls, first in the tensor queue,
            # keep the PE busy (at full duty: FD=512, 4 rotating PSUM banks
            # so no WAW waits) while input DMA streams, so the clock gate
            # un-throttles before/soon-after the real fp8 stream begins.
            # split memset: the first LDWEIGHTS only needs cols 0:128, so it
            # can start while the moving-operand half is still being set
            wu = const.tile([128, 128 + 512], bf16, name="warm")
            nc.vector.memset(wu[:, 0:128], 1.0)
            nc.vector.memset(wu[:, 128:640], 1.0)
            for wi in range(NWARM):
                wps = wpsum.tile([128, 512], f32, tag="wps")
                nc.tensor.matmul(wps[:], lhsT=wu[:, 0:128], rhs=wu[:, 128:640],
                                 start=True, stop=True)

            # --- Input DMA: need-ordered across the three trigger queues
            # (~128KB per trigger: triggers cost ~650ns of issue; the 16 DMA
            # engines fair-share ~300GB/s across queues, first items land
            # ~3.5us after trigger).  sync/scalar are hardware queues (fast
            # ramp); gpsimd is a software queue (slow ramp) and gets the
            # late-needed tiles.
            ehat_sb = [const.tile([128, 8, 128], f8w, tag=f"eh{m}",
                                  name=f"ehat{m}") for m in range(8)]
            g_tiles = {1: gpool.tile([128, 8, CH], f8w, tag="g", name="g1"),
                       2: gpool.tile([128, 8, CH], f8w, tag="g", name="g2")}
            H = 4 * CH
            nc.sync.dma_start(ehat_sb[0][:], EH[:, 0])
            nc.scalar.dma_start(g_tiles[1][:, 0:4, :], GH[:, 0, 0:H])
            nc.gpsimd.dma_start(g_tiles[1][:, 4:8, :], GH[:, 0, H:2 * H])
            nc.sync.dma_start(ehat_sb[2][:], EH[:, 2])
            nc.scalar.dma_start(ehat_sb[1][:], EH[:, 1])
            nc.gpsimd.dma_start(ehat_sb[3][:], EH[:, 3])
            nc.sync.dma_start(ehat_sb[4][:], EH[:, 4])
            nc.scalar.dma_start(ehat_sb[5][:], EH[:, 5])
            nc.gpsimd.dma_start(g_tiles[2][:, 0:4, :], GH[:, 1, 0:H])
            nc.sync.dma_start(ehat_sb[6][:], EH[:, 6])
            nc.scalar.dma_start(ehat_sb[7][:], EH[:, 7])
            nc.gpsimd.dma_start(g_tiles[2][:, 4:8, :], GH[:, 1, H:2 * H])

            # tau 1 moving operand: the G1 tile itself (e4m3 seeds)
            cur_b = [g_tiles[1][:, 0:4, :], g_tiles[1][:, 4:8, :]]
            for tau in range(1, W + 1):
                g_tile = g_tiles.pop(tau)
                if tau + 2 <= W:
                    nt = gpool.tile([128, 8, CH], f8w, tag="g",
                                    name=f"g{tau + 2}")
                    nc.scalar.dma_start(nt[:], GH[:, tau + 1, :])
                    g_tiles[tau + 2] = nt
                new_b = [bpool.tile([128, 4, CH], f8b, tag=f"b{h}",
                                    name=f"b{tau}_{h}") for h in range(2)]
                for mth in range(8):
                    ps = psum.tile([128, CH], f32, tag="ps")
                    for q in range(4):
                        nc.tensor.matmul(
                            ps[:],
                            lhsT=ehat_sb[mth][:, 2 * q:2 * q + 2, :],
                            rhs=cur_b[q // 2][:, 2 * (q % 2):2 * (q % 2) + 2, :],
                            start=(q == 0),
                            stop=(q == 3),
                            perf_mode=DR)
                    nc.vector.tensor_tensor(
                        out=new_b[mth // 4][:, mth % 4, :], in0=ps[:],
                        in1=g_tile[:, mth, :],
                        op=mybir.AluOpType.mult)
                    if tau == V_TAU and mth % 2 == 1 and mth < 6:
                        # dump the 2 just-written j-block slices (64KB) as
                        # soon as their DVE multiplies land; one queue each
                        j = mth // 2
                        eng = [nc.sync, nc.gpsimd, nc.sync][j]
                        eng.dma_start(
                            DV[j],
                            new_b[mth // 4][:, 2 * (j % 2):2 * (j % 2) + 2, :])
                    elif tau == V_TAU and mth >= 6:
                        # the last two slices go out individually (32KB) so
                        # the final dump starts right at the last multiply
                        eng = nc.gpsimd if mth == 6 else nc.scalar
                        eng.dma_start(
                            DV[3, :, (mth - 6) * CH:(mth - 5) * CH],
                            new_b[1][:, mth % 4, :])
                cur_b = [new_b[0][:, :, :], new_b[1][:, :, :]]

    nc.compile()
    return nc


def _get_nc():
    if "nc" not in _CACHE:
        _CACHE["nc"] = _build_nc()
    return _CACHE["nc"]


def _preprocess(emit, trans):
    import ml_dtypes
    f8w = ml_dtypes.float8_e4m3

    emit64 = emit.astype(np.float64)
    trans64 = trans.astype(np.float64)
    Mc = trans64.max(axis=0)
    Ehat = np.exp(trans64 - Mc[None, :]).astype(np.float32)
    # eh[p, mth, jc, q] = Ehat[jc*128+p, mth*128+q]  (partition-major so each
    # EH[:, m] block is one contiguous 128KB DMA)
    eh = np.ascontiguousarray(
        Ehat.reshape(8, 128, 8, 128).transpose(1, 2, 0, 3)
    ).astype(f8w)

    A = emit64 + Mc[None, :]
    mu = A.max(axis=1) + RBAR                       # [T]
    ghat = np.exp(A - mu[:, None]).astype(np.float32)   # [T, NT]

    in_maps = []
    us_all = np.zeros(NCORES * CH)
    for c in range(NCORES):
        # chain i = c*CH + ch covers steps 8i..8i+7 at taus 1..8 (uniform;
        # chain 0's "step 0" slot holds ghat[0], a dummy)
        i0 = c * CH * L
        G = ghat[i0:i0 + CH * L].reshape(CH, W, NT)
        # GH[tau, p, blk*CH+ch] = ghat[step(ch,tau), blk*128+p]
        Gt = (G.transpose(1, 2, 0)                  # [W, NT, CH]
                .reshape(W, 8, 128, CH)
                .transpose(0, 2, 1, 3)
                .reshape(W, 128, 8 * CH))
        gh = np.ascontiguousarray(Gt.transpose(1, 0, 2)).astype(f8w)
        # u_i = e4m3 seed-column sums, from the quantized upload itself
        us_all[c * CH:(c + 1) * CH] = (
            np.asarray(gh)[:, 0, :].astype(np.float64)
            .reshape(128, 8, CH).sum(axis=(0, 1)))
        in_maps.append({"ehat": np.asarray(eh), "ghat": np.asarray(gh)})
    return in_maps, mu, us_all


def _host_alpha7(emit, trans, strans):
    """Exact fp64 forward for steps 0..7 (the dummy chain-0 span)."""
    a = strans.astype(np.float64) + emit[0].astype(np.float64)
    tr = trans.astype(np.float64)
    for t in range(1, L):
        M = a[:, None] + tr
        m = M.max(axis=0)
        a = emit[t].astype(np.float64) + m + np.log(
            np.exp(M - m[None, :]).sum(axis=0))
    return a


def _postprocess(results, alpha7, mu, us_all, etrans):
    """Telescoping seam corrections in fp64, anchored by exact alpha_7."""
    n = NCORES * CH
    Vs = np.zeros(n)
    v_last = None
    for c in range(NCORES):
        dv = (results[c]["dv"].astype(np.float64)
              .reshape(4, 128, 2, CH).transpose(1, 0, 2, 3)
              .reshape(128, 8, CH))
        Vs[c * CH:(c + 1) * CH] = dv.sum(axis=(0, 1))
        if c == NCORES - 1:
            # v[k = blk*128+p] of last chain = dv[p, blk, CH-1]
            v_last = dv[:, :, CH - 1].T.reshape(NT)
    m7 = alpha7.max()
    T0 = m7 + np.log(np.exp(alpha7 - m7).sum())
    Tlast = (T0 + mu[L:].sum()
             + (np.log(Vs[1:]) - np.log(us_all[1:])).sum())
    logZ = (Tlast - np.log(Vs[-1])
            + np.log((v_last * np.exp(etrans.astype(np.float64))).sum()))
    return logZ


def _score(emit, y, trans, strans, etrans):
    y = y.astype(np.int64)
    return (float(strans[y[0]])
            + trans[y[:-1], y[1:]].astype(np.float64).sum()
            + float(etrans[y[-1]])
            + emit[np.arange(T), y].astype(np.float64).sum())


def _ensure_axon_hooks():
    """Some images lack antenv.axon_hooks; bass_utils imports it whenever
    BASS_TRACE is set under axon.  Provide a no-op shim so kernel() never
    crashes on that path (tracing degrades gracefully)."""
    try:
        import antenv.axon_hooks  # noqa: F401
    except ImportError:
        import sys
        import types
        m = types.ModuleType("antenv.axon_hooks")
        state = {"v": None}
        m.get_axon_ntff_profile_hook = lambda: state["v"]
        m.set_axon_ntff_profile_hook = lambda v: state.update(v=v)
        sys.modules["antenv.axon_hooks"] = m


def kernel(emit, y, trans, strans, etrans):
    _ensure_axon_hooks()
    from concourse.bass_utils import run_bass_kernel_spmd

    emit = np.asarray(emit)
    trans = np.asarray(trans)
    strans = np.asarray(strans)
    etrans = np.asarray(etrans)
    y = np.asarray(y)

    nc = _get_nc()
    in_maps, mu, us_all = _preprocess(emit, trans)
    alpha7 = _host_alpha7(emit, trans, strans)
    res = run_bass_kernel_spmd(nc, in_maps, list(range(NCORES)))
    _CACHE["last_res"] = res
    logZ = _postprocess(res.results, alpha7, mu, us_all, etrans)
    out = logZ - _score(emit, y, trans, strans, etrans)
    return np.asarray(out, dtype=np.float32)
